# revision 26
# baseline (speedup 1.0000x reference)
"""Trainium2 Bass kernel for nn_Attention_13348758356565.

Dense transformer attention block (B=16, N=1024 tokens, DIM=1024, 16 heads x 64)
with axial rotary embeddings, data-parallel over batch across 8 NeuronCores
(2 batches per core).

v1 rewrite vs baseline: cross-batch pipelining (batch b+1's QKV matmuls fill
the PE during batch b's ACT-bound attention), exp batched to [128,1024] psum
reads (halves ACT instruction count), bf16 rotary in 3 DVE ops using a
negative-stride pair-swap access pattern with the rotation sign folded into
the sin table, bf16 PE transposes, and a PSUM budget of exactly 8 banks
(st 4 + pv 2 + matmul-stream 2).
"""

import os
import sys

sys.path.insert(0, "/opt/trn_rl_repo")

import dataclasses
import numpy as np

import concourse.bacc as bacc
import concourse.mybir as mybir
import concourse.tile as tile
from concourse import bass_utils

F32 = mybir.dt.float32
F32R = mybir.dt.float32r
BF16 = mybir.dt.bfloat16
EXP = mybir.ActivationFunctionType.Exp

B, HF, WF = 16, 32, 32
DIM, NH, HD = 1024, 16, 64
N = HF * WF          # 1024 tokens
NCORES = 8
BPC = B // NCORES    # 2 batches per core
ROT = HD // 2        # 32 rotary dims per head
SCALE = 1.0 / np.sqrt(HD)

NT = N // 128        # 8 token tiles
ND = DIM // 128      # 8 contraction tiles
NP = NH // 2         # 8 head pairs

last_exec_time_ns = None


def _round13(x):
    """Round fp32 mantissa to 13 bits (safe operand form for f32r matmuls)."""
    xi = np.ascontiguousarray(x, np.float32).view(np.uint32)
    xi = ((xi.astype(np.uint64) + (1 << 9)) >> 10 << 10).astype(np.uint32)
    return xi.view(np.float32)


def _freq_tables():
    """cos/sin expanded to the 32 rotary dims with rotate-half sign folded in.

    C2[n, k] = cos(f[n, k//2]);  S2[n, 2i] = -sin(f[n, i]), S2[n, 2i+1] = +sin.
    Rotation: out[k] = q[k]*C2[k] + q[k^1]*S2[k].
    """
    d = HD // 4
    base = (np.linspace(1.0, (HF * WF) / 2.0, d // 2, dtype=np.float64) * np.pi)
    posH = np.linspace(-1.0, 1.0, HF)
    posW = np.linspace(-1.0, 1.0, WF)
    fH = np.repeat(posH[:, None] * base[None, :], 2, axis=-1)   # [H, 16]
    fW = np.repeat(posW[:, None] * base[None, :], 2, axis=-1)   # [W, 16]
    fH = np.broadcast_to(fH[:, None, :], (HF, WF, d))
    fW = np.broadcast_to(fW[None, :, :], (HF, WF, d))
    freqs = np.concatenate([fH, fW], axis=-1).reshape(N, ROT)
    half = freqs[:, 0::2]                                       # [N, 16] pairs
    cos_h = np.cos(half)
    sin_h = np.sin(half)
    C2 = np.repeat(cos_h, 2, axis=1)                            # [N, 32]
    S2 = np.empty((N, ROT), np.float64)
    S2[:, 0::2] = -sin_h
    S2[:, 1::2] = sin_h
    return C2.astype(np.float32), S2.astype(np.float32)


def _bcast_mid(ap, count):
    """Insert a step-0 (broadcast) dim into AP after the partition dim."""
    return dataclasses.replace(ap, ap=[ap.ap[0], [0, count]] + list(ap.ap[1:]))


def _build():
    nc = bacc.Bacc("TRN2", target_bir_lowering=False, debug=False)

    xT_d = nc.dram_tensor("xT", [BPC, DIM, N], F32R, kind="ExternalInput")
    wqkvT_d = nc.dram_tensor("wqkvT", [DIM, 3 * DIM], F32R, kind="ExternalInput")
    wprojT_d = nc.dram_tensor("wprojT", [DIM, DIM], BF16, kind="ExternalInput")
    bproj_d = nc.dram_tensor("bproj", [1, DIM], BF16, kind="ExternalInput")
    cos2_d = nc.dram_tensor("cos2", [N, ROT], BF16, kind="ExternalInput")
    sin2_d = nc.dram_tensor("sin2", [N, ROT], BF16, kind="ExternalInput")
    ident_d = nc.dram_tensor("ident", [128, 128], BF16, kind="ExternalInput")
    ones_d = nc.dram_tensor("ones", [1, 128], BF16, kind="ExternalInput")
    y_d = nc.dram_tensor("y", [BPC, N, DIM], F32, kind="ExternalOutput")

    mul = mybir.AluOpType.mult
    add = mybir.AluOpType.add

    with tile.TileContext(nc) as tc:
        with (
            tc.tile_pool(name="sb", bufs=1) as sb,
            tc.tile_pool(name="ps", bufs=1, space="PSUM") as ps,
        ):
            # ---- constants ----
            ident = sb.tile([128, 128], BF16, name="ident")
            nc.sync.dma_start(ident[:], ident_d.ap())
            ones_r = sb.tile([1, 128], BF16, name="ones_r")
            nc.sync.dma_start(ones_r[:], ones_d.ap())
            bproj = sb.tile([1, DIM], BF16, name="bproj")
            nc.sync.dma_start(bproj[:], bproj_d.ap())
            # rotary tables: [128 tok-in-tile, t, 32]
            cos2 = sb.tile([128, NT * ROT], BF16, name="cos2")
            sin2 = sb.tile([128, NT * ROT], BF16, name="sin2")
            nc.sync.dma_start(
                cos2[:].rearrange("p (t c) -> p t c", c=ROT),
                cos2_d.ap().rearrange("(t p) c -> p t c", p=128),
            )
            nc.sync.dma_start(
                sin2[:].rearrange("p (t c) -> p t c", c=ROT),
                sin2_d.ap().rearrange("(t p) c -> p t c", p=128),
            )
            # wproj isn't needed until the first projection (~200us in); load
            # late so the startup DMA window is spent on x and w_qkv
            wprojT = [sb.tile([128, DIM], BF16, name=f"wprojT{d}") for d in range(ND)]
            wproj_loaded = [False]

            def load_wproj():
                if not wproj_loaded[0]:
                    wproj_loaded[0] = True
                    for d in range(ND):
                        nc.sync.dma_start(
                            wprojT[d][:], wprojT_d.ap()[d * 128:(d + 1) * 128, :])

            # HAM warm-up: ~5us of back-to-back dummy matmuls during the
            # initial x/w DMA wait so real work starts at the 2.4 GHz clock
            warm = ps.tile([128, 128], F32, name="warm", tag="mmq", bufs=2)
            for _ in range(48):
                nc.tensor.matmul(warm[:], ident[:], ident[:])

            def qkv_chunk(b, xT, tag, j0, consume):
                """qkv[:, j0:j0+512] for batch b; consume(t, pq) eats each
                [128, 512] psum tile of token-tile t."""
                wq = [sb.tile([128, 512], F32R, name=f"wq_{tag}_{d}",
                              tag=f"wq{d}", bufs=2) for d in range(ND)]
                for d in range(ND):
                    nc.sync.dma_start(
                        wq[d][:], wqkvT_d.ap()[d * 128:(d + 1) * 128, j0:j0 + 512])
                for t in range(NT):
                    pq = ps.tile([128, 512], F32, name=f"pq_{tag}_{t}",
                                 tag="mmq", bufs=2)
                    for d in range(ND):
                        nc.tensor.matmul(
                            pq[:], xT[d][:, t * 128:(t + 1) * 128], wq[d][:],
                            start=(d == 0), stop=(d == ND - 1))
                    consume(t, pq)

            def rotary(tag, t, pq):
                """Evacuate psum -> bf16 sbuf and rotate first 32 dims of each
                of the 8 heads in place: out[k] = q[k]*C2 + q[k^1]*S2."""
                qn = sb.tile([128, 512], BF16, name=f"qn_{tag}_{t}",
                             tag="qn", bufs=8)
                nc.vector.tensor_copy(qn[:], pq[:])   # frees the psum slot
                v4 = qn[:].rearrange("p (h i u) -> p h i u", h=8, i=32, u=2)
                rot = v4[:, :, 0:16, :]               # [128, 8, 16, 2]
                swp = v4[:, :, 0:16, ::-1]            # pair-swapped read
                cb = _bcast_mid(
                    cos2[:, t * ROT:(t + 1) * ROT]
                    .rearrange("p (i u) -> p i u", u=2), 8)
                sbb = _bcast_mid(
                    sin2[:, t * ROT:(t + 1) * ROT]
                    .rearrange("p (i u) -> p i u", u=2), 8)
                t1 = sb.tile([128, 8, 16, 2], BF16, name=f"t1_{tag}_{t}",
                             tag="rt1", bufs=2)
                t2 = sb.tile([128, 8, 16, 2], BF16, name=f"t2_{tag}_{t}",
                             tag="rt2", bufs=2)
                nc.vector.tensor_tensor(t1[:], rot, cb, mul)
                nc.vector.tensor_tensor(t2[:], swp, sbb, mul)
                nc.vector.tensor_tensor(rot, t1[:], t2[:], add)
                return qn

            def transpose_group(tag, qn4, grp, jt, dst):
                """PE-transpose col jt of 4 natural tiles into dst[:, grp*512:]."""
                tp = ps.tile([128, 512], BF16, name=f"tp_{tag}_{jt}_{grp}",
                             tag="mmq", bufs=2)
                for u in range(4):
                    nc.tensor.transpose(
                        tp[:, u * 128:(u + 1) * 128],
                        qn4[u][:, jt * 128:(jt + 1) * 128], ident[:])
                nc.vector.tensor_copy(dst[:, grp * 512:(grp + 1) * 512], tp[:])

            pending = []

            def flush_pending():
                for fn in pending:
                    fn()
                pending.clear()

            def attention(b, hp, kT, qT, vsb, outT):
                """One head pair: scores in [128,1024] psum (A|B), one exp,
                PV with ones-column denominators, normalize into outT.

                The last LAG PV matmuls and the normalize chain are deferred
                into the next (hp, nch) block so the next pair's independent
                QK matmuls sit between them in the PE queue instead of the PE
                head-of-line blocking on the tail exps."""
                LAG = 2
                hA, hB = 2 * hp, 2 * hp + 1
                for nch in range(2):
                    box = {}
                    pts = {}

                    def emit_pv(m, box=box, pts=pts, b=b, hp=hp, nch=nch,
                                hA=hA, hB=hB, vsb=vsb):
                        if "pvA" not in box:
                            box["pvA"] = ps.tile(
                                [HD + 1, 512], F32, name=f"pvA_b{b}_{hp}_{nch}",
                                tag="pv", bufs=2)
                            box["pvB"] = ps.tile(
                                [HD + 1, 512], F32, name=f"pvB_b{b}_{hp}_{nch}",
                                tag="pv", bufs=2)
                        pt = pts.pop(m)
                        nc.tensor.matmul(
                            box["pvA"][:], vsb[m][:, hA * (HD + 1):(hA + 1) * (HD + 1)],
                            pt[:, 0:512], start=(m == 0), stop=(m == NT - 1))
                        nc.tensor.matmul(
                            box["pvB"][:], vsb[m][:, hB * (HD + 1):(hB + 1) * (HD + 1)],
                            pt[:, 512:1024], start=(m == 0), stop=(m == NT - 1))

                    def normalize(box=box, b=b, hp=hp, nch=nch, outT=outT):
                        for half, pvx in ((0, box["pvA"]), (1, box["pvB"])):
                            # evacuate psum fast (frees the pv bank for the
                            # next pair), then normalize from sbuf
                            pvE = sb.tile([HD + 1, 512], F32,
                                          name=f"pvE_b{b}_{hp}_{nch}_{half}",
                                          tag="pvE", bufs=2)
                            nc.vector.tensor_copy(pvE[:], pvx[:])
                            dr = sb.tile([1, 512], F32,
                                         name=f"dr_b{b}_{hp}_{nch}_{half}",
                                         tag="dr", bufs=2)
                            nc.vector.tensor_copy(dr[:], pvE[64:65, :])
                            rr = sb.tile([1, 512], F32,
                                         name=f"rr_b{b}_{hp}_{nch}_{half}",
                                         tag="rr", bufs=2)
                            nc.vector.reciprocal_approx_fast(rr[:], dr[:])
                            rb = sb.tile([64, 512], F32,
                                         name=f"rb_b{b}_{hp}_{nch}_{half}",
                                         tag="rb", bufs=2)
                            nc.gpsimd.partition_broadcast(rb[:], rr[:])
                            nc.vector.tensor_tensor(
                                outT[hp][half * 64:half * 64 + 64,
                                         nch * 512:(nch + 1) * 512],
                                pvE[0:64, :], rb[:], mul)

                    for m in range(NT):
                        st = ps.tile([128, 1024], F32, name=f"st_b{b}_{hp}_{nch}_{m}",
                                     tag="st", bufs=2)
                        nc.tensor.matmul(
                            st[:, 0:512],
                            kT[hp][0:64, m * 128:(m + 1) * 128],
                            qT[hp][0:64, nch * 512:(nch + 1) * 512])
                        nc.tensor.matmul(
                            st[:, 512:1024],
                            kT[hp][64:128, m * 128:(m + 1) * 128],
                            qT[hp][64:128, nch * 512:(nch + 1) * 512])
                        pt = sb.tile([128, 1024], BF16, name=f"pt_b{b}_{hp}_{nch}_{m}",
                                     tag="pt", bufs=6)
                        nc.scalar.activation(pt[:], st[:], EXP, scale=float(SCALE))
                        pts[m] = pt
                        if m == 0:
                            flush_pending()
                        if m >= LAG:
                            emit_pv(m - LAG)
                    for m in range(NT - LAG, NT):
                        pending.append(lambda m=m, emit_pv=emit_pv: emit_pv(m))
                    pending.append(normalize)

            for b in range(BPC):
                xT = []
                for d in range(ND):
                    xt = sb.tile([128, N], F32R, name=f"xT_b{b}_{d}", tag=f"xT{d}")
                    nc.sync.dma_start(xt[:], xT_d.ap()[b, d * 128:(d + 1) * 128, :])
                    xT.append(xt)

                vsb = [sb.tile([128, NH * (HD + 1)], BF16, name=f"v_b{b}_{t}",
                               tag=f"v{t}", bufs=2) for t in range(NT)]
                for t in range(NT):
                    nc.vector.memset(
                        vsb[t][:].rearrange("p (h c) -> p h c", c=HD + 1)
                        [:, :, HD:], 1.0)

                qT = [sb.tile([128, N], BF16, name=f"qT_b{b}_{p}", tag=f"qT{p}")
                      for p in range(NP)]
                kT = [sb.tile([128, N], BF16, name=f"kT_b{b}_{p}", tag=f"kT{p}")
                      for p in range(NP)]
                outT = [sb.tile([128, N], BF16, name=f"outT_b{b}_{p}",
                                tag=f"outT{p}") for p in range(NP)]

                # ---- V first (it gates every head's PV) ----
                for jc in range(2):
                    def eat_v(t, pq, jc=jc):
                        h0 = jc * 8
                        nc.vector.tensor_copy(
                            vsb[t][:].rearrange("p (h c) -> p h c", c=HD + 1)
                            [:, h0:h0 + 8, 0:HD],
                            pq[:].rearrange("p (h c) -> p h c", c=HD))
                    qkv_chunk(b, xT, f"v{b}{jc}", 2 * DIM + jc * 512, eat_v)

                # ---- per group of 4 pairs: K chunk, Q chunk, attention ----
                for g in range(2):
                    for sect, dst_all in ((1, kT), (0, qT)):
                        qn_tiles = []

                        def eat_qk(t, pq, sect=sect, g=g, qn_tiles=qn_tiles):
                            qn_tiles.append(rotary(f"s{sect}b{b}g{g}", t, pq))
                            if t % 4 == 3:
                                grp = t // 4
                                for jt in range(4):
                                    transpose_group(
                                        f"s{sect}b{b}g{g}", qn_tiles[grp * 4:],
                                        grp, jt, dst_all[4 * g + jt])
                        qkv_chunk(b, xT, f"s{sect}b{b}g{g}",
                                  sect * DIM + g * 512, eat_qk)
                    for hp in range(4 * g, 4 * g + 4):
                        attention(b, hp, kT, qT, vsb, outT)

                flush_pending()
                load_wproj()

                # ---- output projection ----
                for t in range(NT):
                    for ec in range(2):
                        py = ps.tile([128, 512], F32, name=f"py_b{b}_{t}_{ec}",
                                     tag="mmq", bufs=2)
                        for d in range(ND):
                            nc.tensor.matmul(
                                py[:],
                                outT[d][:, t * 128:(t + 1) * 128],
                                wprojT[d][:, ec * 512:(ec + 1) * 512],
                                start=(d == 0), stop=False,
                            )
                        nc.tensor.matmul(
                            py[:], ones_r[:], bproj[:, ec * 512:(ec + 1) * 512],
                            start=False, stop=True,
                        )
                        ysb = sb.tile([128, 512], F32, name=f"y_b{b}_{t}_{ec}",
                                      tag="ysb", bufs=2)
                        nc.vector.tensor_copy(ysb[:], py[:])
                        nc.sync.dma_start(
                            y_d.ap()[b, t * 128:(t + 1) * 128,
                                     ec * 512:(ec + 1) * 512],
                            ysb[:],
                        )

    nc.compile()
    return nc


_NC_CACHE = None


def kernel(x, w_qkv, w_proj, b_proj):
    global _NC_CACHE, last_exec_time_ns
    x = np.ascontiguousarray(np.asarray(x, np.float32))
    w_qkv = np.asarray(w_qkv, np.float32)
    w_proj = np.asarray(w_proj, np.float32)
    b_proj = np.asarray(b_proj, np.float32)

    if _NC_CACHE is None:
        _NC_CACHE = _build()
    nc = _NC_CACHE

    import ml_dtypes
    C2, S2 = _freq_tables()
    cos2 = C2.astype(ml_dtypes.bfloat16)
    sin2 = S2.astype(ml_dtypes.bfloat16)
    wqkvT = _round13(np.ascontiguousarray(w_qkv.T))
    wprojT16 = np.ascontiguousarray(w_proj.T).astype(ml_dtypes.bfloat16)
    bproj16 = b_proj.reshape(1, DIM).astype(ml_dtypes.bfloat16)
    ones16 = np.ones((1, 128), ml_dtypes.bfloat16)
    ident = np.eye(128, dtype=np.float32).astype(ml_dtypes.bfloat16)

    in_maps = []
    for c in range(NCORES):
        xs = x[c * BPC:(c + 1) * BPC]                       # [2, N, DIM]
        xT = _round13(np.ascontiguousarray(xs.transpose(0, 2, 1)))
        in_maps.append({
            "xT": xT, "wqkvT": wqkvT, "wprojT": wprojT16,
            "bproj": bproj16, "cos2": cos2, "sin2": sin2,
            "ident": ident, "ones": ones16,
        })

    trace = bool(os.environ.get("KERNEL_TRACE"))
    kwargs = {}
    if trace:
        kwargs["trace"] = True
        td = os.environ.get("KERNEL_TRACE_DIR")
        if td:
            kwargs["tmpdir"] = td
    res = bass_utils.run_bass_kernel_spmd(
        nc, in_maps, core_ids=list(range(NCORES)), **kwargs)
    last_exec_time_ns = res.exec_time_ns
    out = np.concatenate([res.results[c]["y"] for c in range(NCORES)], axis=0)
    return np.ascontiguousarray(out.reshape(B, N, DIM).astype(np.float32))


if __name__ == "__main__":
    rng = np.random.default_rng(0)
    xs = rng.standard_normal((B, N, DIM), dtype=np.float32)
    wq = rng.standard_normal((3 * DIM, DIM), dtype=np.float32) / 32
    wp = rng.standard_normal((DIM, DIM), dtype=np.float32) / 32
    bp = np.zeros(DIM, np.float32)
    y = kernel(xs, wq, wp, bp)
    print("y", y.shape, y.dtype, float(np.abs(y).max()))


# revision 28
# speedup vs baseline: 1.0066x; 1.0066x over previous
"""Trainium2 Bass kernel for nn_Attention_13348758356565.

Dense transformer attention block (B=16, N=1024 tokens, DIM=1024, 16 heads x 64)
with axial rotary embeddings, data-parallel over batch across 8 NeuronCores
(2 batches per core).

v1 rewrite vs baseline: cross-batch pipelining (batch b+1's QKV matmuls fill
the PE during batch b's ACT-bound attention), exp batched to [128,1024] psum
reads (halves ACT instruction count), bf16 rotary in 3 DVE ops using a
negative-stride pair-swap access pattern with the rotation sign folded into
the sin table, bf16 PE transposes, and a PSUM budget of exactly 8 banks
(st 4 + pv 2 + matmul-stream 2).
"""

import os
import sys

sys.path.insert(0, "/opt/trn_rl_repo")

import dataclasses
import numpy as np

import concourse.bacc as bacc
import concourse.mybir as mybir
import concourse.tile as tile
from concourse import bass_utils

F32 = mybir.dt.float32
F32R = mybir.dt.float32r
BF16 = mybir.dt.bfloat16
EXP = mybir.ActivationFunctionType.Exp

B, HF, WF = 16, 32, 32
DIM, NH, HD = 1024, 16, 64
N = HF * WF          # 1024 tokens
NCORES = 8
BPC = B // NCORES    # 2 batches per core
ROT = HD // 2        # 32 rotary dims per head
SCALE = 1.0 / np.sqrt(HD)

NT = N // 128        # 8 token tiles
ND = DIM // 128      # 8 contraction tiles
NP = NH // 2         # 8 head pairs

last_exec_time_ns = None


def _round13(x):
    """Round fp32 mantissa to 13 bits (safe operand form for f32r matmuls)."""
    xi = np.ascontiguousarray(x, np.float32).view(np.uint32)
    xi = ((xi.astype(np.uint64) + (1 << 9)) >> 10 << 10).astype(np.uint32)
    return xi.view(np.float32)


def _freq_tables():
    """cos/sin expanded to the 32 rotary dims with rotate-half sign folded in.

    C2[n, k] = cos(f[n, k//2]);  S2[n, 2i] = -sin(f[n, i]), S2[n, 2i+1] = +sin.
    Rotation: out[k] = q[k]*C2[k] + q[k^1]*S2[k].
    """
    d = HD // 4
    base = (np.linspace(1.0, (HF * WF) / 2.0, d // 2, dtype=np.float64) * np.pi)
    posH = np.linspace(-1.0, 1.0, HF)
    posW = np.linspace(-1.0, 1.0, WF)
    fH = np.repeat(posH[:, None] * base[None, :], 2, axis=-1)   # [H, 16]
    fW = np.repeat(posW[:, None] * base[None, :], 2, axis=-1)   # [W, 16]
    fH = np.broadcast_to(fH[:, None, :], (HF, WF, d))
    fW = np.broadcast_to(fW[None, :, :], (HF, WF, d))
    freqs = np.concatenate([fH, fW], axis=-1).reshape(N, ROT)
    half = freqs[:, 0::2]                                       # [N, 16] pairs
    cos_h = np.cos(half)
    sin_h = np.sin(half)
    C2 = np.repeat(cos_h, 2, axis=1)                            # [N, 32]
    S2 = np.empty((N, ROT), np.float64)
    S2[:, 0::2] = -sin_h
    S2[:, 1::2] = sin_h
    return C2.astype(np.float32), S2.astype(np.float32)


def _bcast_mid(ap, count):
    """Insert a step-0 (broadcast) dim into AP after the partition dim."""
    return dataclasses.replace(ap, ap=[ap.ap[0], [0, count]] + list(ap.ap[1:]))


def _build():
    nc = bacc.Bacc("TRN2", target_bir_lowering=False, debug=False)

    xT_d = nc.dram_tensor("xT", [BPC, DIM, N], F32R, kind="ExternalInput")
    wqkvT_d = nc.dram_tensor("wqkvT", [DIM, 3 * DIM], F32R, kind="ExternalInput")
    wprojT_d = nc.dram_tensor("wprojT", [DIM, DIM], BF16, kind="ExternalInput")
    bproj_d = nc.dram_tensor("bproj", [1, DIM], BF16, kind="ExternalInput")
    cos2_d = nc.dram_tensor("cos2", [N, ROT], BF16, kind="ExternalInput")
    sin2_d = nc.dram_tensor("sin2", [N, ROT], BF16, kind="ExternalInput")
    ident_d = nc.dram_tensor("ident", [128, 128], BF16, kind="ExternalInput")
    ones_d = nc.dram_tensor("ones", [1, 128], BF16, kind="ExternalInput")
    y_d = nc.dram_tensor("y", [BPC, N, DIM], F32, kind="ExternalOutput")

    mul = mybir.AluOpType.mult
    add = mybir.AluOpType.add

    with tile.TileContext(nc) as tc:
        with (
            tc.tile_pool(name="sb", bufs=1) as sb,
            tc.tile_pool(name="ps", bufs=1, space="PSUM") as ps,
        ):
            # ---- constants ----
            ident = sb.tile([128, 128], BF16, name="ident")
            nc.sync.dma_start(ident[:], ident_d.ap())
            ones_r = sb.tile([1, 128], BF16, name="ones_r")
            nc.sync.dma_start(ones_r[:], ones_d.ap())
            bproj = sb.tile([1, DIM], BF16, name="bproj")
            nc.sync.dma_start(bproj[:], bproj_d.ap())
            # rotary tables: [128 tok-in-tile, t, 32]
            cos2 = sb.tile([128, NT * ROT], BF16, name="cos2")
            sin2 = sb.tile([128, NT * ROT], BF16, name="sin2")
            nc.sync.dma_start(
                cos2[:].rearrange("p (t c) -> p t c", c=ROT),
                cos2_d.ap().rearrange("(t p) c -> p t c", p=128),
            )
            nc.sync.dma_start(
                sin2[:].rearrange("p (t c) -> p t c", c=ROT),
                sin2_d.ap().rearrange("(t p) c -> p t c", p=128),
            )
            # wproj isn't needed until the first projection (~200us in); load
            # late so the startup DMA window is spent on x and w_qkv
            wprojT = [sb.tile([128, DIM], BF16, name=f"wprojT{d}") for d in range(ND)]
            wproj_loaded = [False]

            def load_wproj():
                if not wproj_loaded[0]:
                    wproj_loaded[0] = True
                    for d in range(ND):
                        nc.sync.dma_start(
                            wprojT[d][:], wprojT_d.ap()[d * 128:(d + 1) * 128, :])

            # HAM warm-up: ~5us of back-to-back dummy matmuls during the
            # initial x/w DMA wait so real work starts at the 2.4 GHz clock
            warm = ps.tile([128, 128], F32, name="warm", tag="mmq", bufs=2)
            for _ in range(48):
                nc.tensor.matmul(warm[:], ident[:], ident[:])

            def qkv_chunk(b, xT, tag, j0, consume):
                """qkv[:, j0:j0+512] for batch b; consume(t, pq) eats each
                [128, 512] psum tile of token-tile t."""
                wq = [sb.tile([128, 512], F32R, name=f"wq_{tag}_{d}",
                              tag=f"wq{d}", bufs=2) for d in range(ND)]
                for d in range(ND):
                    nc.sync.dma_start(
                        wq[d][:], wqkvT_d.ap()[d * 128:(d + 1) * 128, j0:j0 + 512])
                for t in range(NT):
                    pq = ps.tile([128, 512], F32, name=f"pq_{tag}_{t}",
                                 tag="mmq", bufs=2)
                    for d in range(ND):
                        nc.tensor.matmul(
                            pq[:], xT[d][:, t * 128:(t + 1) * 128], wq[d][:],
                            start=(d == 0), stop=(d == ND - 1))
                    consume(t, pq)

            def rotary(tag, t, pq):
                """Evacuate psum -> bf16 sbuf and rotate first 32 dims of each
                of the 8 heads in place: out[k] = q[k]*C2 + q[k^1]*S2."""
                qn = sb.tile([128, 512], BF16, name=f"qn_{tag}_{t}",
                             tag="qn", bufs=8)
                nc.vector.tensor_copy(qn[:], pq[:])   # frees the psum slot
                v4 = qn[:].rearrange("p (h i u) -> p h i u", h=8, i=32, u=2)
                rot = v4[:, :, 0:16, :]               # [128, 8, 16, 2]
                swp = v4[:, :, 0:16, ::-1]            # pair-swapped read
                cb = _bcast_mid(
                    cos2[:, t * ROT:(t + 1) * ROT]
                    .rearrange("p (i u) -> p i u", u=2), 8)
                sbb = _bcast_mid(
                    sin2[:, t * ROT:(t + 1) * ROT]
                    .rearrange("p (i u) -> p i u", u=2), 8)
                t1 = sb.tile([128, 8, 16, 2], BF16, name=f"t1_{tag}_{t}",
                             tag="rt1", bufs=2)
                t2 = sb.tile([128, 8, 16, 2], BF16, name=f"t2_{tag}_{t}",
                             tag="rt2", bufs=2)
                nc.vector.tensor_tensor(t1[:], rot, cb, mul)
                nc.vector.tensor_tensor(t2[:], swp, sbb, mul)
                nc.vector.tensor_tensor(rot, t1[:], t2[:], add)
                return qn

            def transpose_group(tag, qn4, grp, jt, dst):
                """PE-transpose col jt of 4 natural tiles into dst[:, grp*512:]."""
                tp = ps.tile([128, 512], BF16, name=f"tp_{tag}_{jt}_{grp}",
                             tag="mmq", bufs=2)
                for u in range(4):
                    nc.tensor.transpose(
                        tp[:, u * 128:(u + 1) * 128],
                        qn4[u][:, jt * 128:(jt + 1) * 128], ident[:])
                nc.vector.tensor_copy(dst[:, grp * 512:(grp + 1) * 512], tp[:])

            pending = []

            def flush_pending():
                for fn in pending:
                    fn()
                pending.clear()

            def attention(b, hp, kT, qT, vsb, outT):
                """One head pair: scores in [128,1024] psum (A|B), one exp,
                PV with ones-column denominators, normalize into outT.

                The last LAG PV matmuls and the normalize chain are deferred
                into the next (hp, nch) block so the next pair's independent
                QK matmuls sit between them in the PE queue instead of the PE
                head-of-line blocking on the tail exps."""
                LAG = 2
                hA, hB = 2 * hp, 2 * hp + 1
                for nch in range(2):
                    box = {}
                    pts = {}

                    def emit_pv(m, box=box, pts=pts, b=b, hp=hp, nch=nch,
                                hA=hA, hB=hB, vsb=vsb):
                        if "pvA" not in box:
                            box["pvA"] = ps.tile(
                                [HD + 1, 512], F32, name=f"pvA_b{b}_{hp}_{nch}",
                                tag="pv", bufs=2)
                            box["pvB"] = ps.tile(
                                [HD + 1, 512], F32, name=f"pvB_b{b}_{hp}_{nch}",
                                tag="pv", bufs=2)
                        pt = pts.pop(m)
                        nc.tensor.matmul(
                            box["pvA"][:], vsb[m][:, hA * (HD + 1):(hA + 1) * (HD + 1)],
                            pt[:, 0:512], start=(m == 0), stop=(m == NT - 1))
                        nc.tensor.matmul(
                            box["pvB"][:], vsb[m][:, hB * (HD + 1):(hB + 1) * (HD + 1)],
                            pt[:, 512:1024], start=(m == 0), stop=(m == NT - 1))

                    def normalize(box=box, b=b, hp=hp, nch=nch, outT=outT):
                        for half, pvx in ((0, box["pvA"]), (1, box["pvB"])):
                            # evacuate psum fast (frees the pv bank for the
                            # next pair), then normalize from sbuf
                            pvE = sb.tile([HD + 1, 512], F32,
                                          name=f"pvE_b{b}_{hp}_{nch}_{half}",
                                          tag="pvE", bufs=2)
                            nc.vector.tensor_copy(pvE[:], pvx[:])
                            dr = sb.tile([1, 512], F32,
                                         name=f"dr_b{b}_{hp}_{nch}_{half}",
                                         tag="dr", bufs=2)
                            nc.vector.tensor_copy(dr[:], pvE[64:65, :])
                            rr = sb.tile([1, 512], F32,
                                         name=f"rr_b{b}_{hp}_{nch}_{half}",
                                         tag="rr", bufs=2)
                            nc.vector.reciprocal_approx_fast(rr[:], dr[:])
                            rb = sb.tile([64, 512], F32,
                                         name=f"rb_b{b}_{hp}_{nch}_{half}",
                                         tag="rb", bufs=2)
                            nc.gpsimd.partition_broadcast(rb[:], rr[:])
                            nc.vector.tensor_tensor(
                                outT[hp][half * 64:half * 64 + 64,
                                         nch * 512:(nch + 1) * 512],
                                pvE[0:64, :], rb[:], mul)

                    for m in range(NT):
                        st = ps.tile([128, 1024], F32, name=f"st_b{b}_{hp}_{nch}_{m}",
                                     tag="st", bufs=2)
                        nc.tensor.matmul(
                            st[:, 0:512],
                            kT[hp][0:64, m * 128:(m + 1) * 128],
                            qT[hp][0:64, nch * 512:(nch + 1) * 512])
                        nc.tensor.matmul(
                            st[:, 512:1024],
                            kT[hp][64:128, m * 128:(m + 1) * 128],
                            qT[hp][64:128, nch * 512:(nch + 1) * 512])
                        pt = sb.tile([128, 1024], BF16, name=f"pt_b{b}_{hp}_{nch}_{m}",
                                     tag="pt", bufs=6)
                        nc.scalar.activation(pt[:], st[:], EXP, scale=float(SCALE))
                        pts[m] = pt
                        if m == 0:
                            flush_pending()
                        if m >= LAG:
                            emit_pv(m - LAG)
                    for m in range(NT - LAG, NT):
                        pending.append(lambda m=m, emit_pv=emit_pv: emit_pv(m))
                    pending.append(normalize)

            for b in range(BPC):
                xT = []
                for d in range(ND):
                    xt = sb.tile([128, N], F32R, name=f"xT_b{b}_{d}", tag=f"xT{d}")
                    nc.sync.dma_start(xt[:], xT_d.ap()[b, d * 128:(d + 1) * 128, :])
                    xT.append(xt)

                vsb = [sb.tile([128, NH * (HD + 1)], BF16, name=f"v_b{b}_{t}",
                               tag=f"v{t}", bufs=2) for t in range(NT)]
                for t in range(NT):
                    nc.vector.memset(
                        vsb[t][:].rearrange("p (h c) -> p h c", c=HD + 1)
                        [:, :, HD:], 1.0)

                qT = [sb.tile([128, N], BF16, name=f"qT_b{b}_{p}", tag=f"qT{p}")
                      for p in range(NP)]
                kT = [sb.tile([128, N], BF16, name=f"kT_b{b}_{p}", tag=f"kT{p}")
                      for p in range(NP)]
                outT = [sb.tile([128, N], BF16, name=f"outT_b{b}_{p}",
                                tag=f"outT{p}") for p in range(NP)]

                # ---- V first (it gates every head's PV) ----
                for jc in range(2):
                    def eat_v(t, pq, jc=jc):
                        h0 = jc * 8
                        nc.vector.tensor_copy(
                            vsb[t][:].rearrange("p (h c) -> p h c", c=HD + 1)
                            [:, h0:h0 + 8, 0:HD],
                            pq[:].rearrange("p (h c) -> p h c", c=HD))
                    qkv_chunk(b, xT, f"v{b}{jc}", 2 * DIM + jc * 512, eat_v)

                # ---- per group of 4 pairs: K chunk, Q chunk, attention ----
                for g in range(2):
                    for sect, dst_all in ((1, kT), (0, qT)):
                        qn_tiles = []

                        def eat_qk(t, pq, sect=sect, g=g, qn_tiles=qn_tiles):
                            qn_tiles.append(rotary(f"s{sect}b{b}g{g}", t, pq))
                            if t % 4 == 3:
                                grp = t // 4
                                for jt in range(4):
                                    transpose_group(
                                        f"s{sect}b{b}g{g}", qn_tiles[grp * 4:],
                                        grp, jt, dst_all[4 * g + jt])
                        qkv_chunk(b, xT, f"s{sect}b{b}g{g}",
                                  sect * DIM + g * 512, eat_qk)
                    for hp in range(4 * g, 4 * g + 4):
                        attention(b, hp, kT, qT, vsb, outT)

                flush_pending()
                load_wproj()

                # ---- output projection ----
                for t in range(NT):
                    for ec in range(2):
                        py = ps.tile([128, 512], F32, name=f"py_b{b}_{t}_{ec}",
                                     tag="mmq", bufs=2)
                        for d in range(ND):
                            nc.tensor.matmul(
                                py[:],
                                outT[d][:, t * 128:(t + 1) * 128],
                                wprojT[d][:, ec * 512:(ec + 1) * 512],
                                start=(d == 0), stop=False,
                            )
                        nc.tensor.matmul(
                            py[:], ones_r[:], bproj[:, ec * 512:(ec + 1) * 512],
                            start=False, stop=True,
                        )
                        ysb = sb.tile([128, 512], F32, name=f"y_b{b}_{t}_{ec}",
                                      tag="ysb", bufs=2)
                        nc.vector.tensor_copy(ysb[:], py[:])
                        nc.sync.dma_start(
                            y_d.ap()[b, t * 128:(t + 1) * 128,
                                     ec * 512:(ec + 1) * 512],
                            ysb[:],
                        )

    nc.compile()
    return nc


_NC_CACHE = None


def kernel(x, w_qkv, w_proj, b_proj):
    global _NC_CACHE, last_exec_time_ns
    x = np.ascontiguousarray(np.asarray(x, np.float32))
    w_qkv = np.asarray(w_qkv, np.float32)
    w_proj = np.asarray(w_proj, np.float32)
    b_proj = np.asarray(b_proj, np.float32)

    if _NC_CACHE is None:
        _NC_CACHE = _build()
    nc = _NC_CACHE

    import ml_dtypes
    C2, S2 = _freq_tables()
    cos2 = C2.astype(ml_dtypes.bfloat16)
    sin2 = S2.astype(ml_dtypes.bfloat16)
    wqkvT = _round13(np.ascontiguousarray(w_qkv.T))
    wprojT16 = np.ascontiguousarray(w_proj.T).astype(ml_dtypes.bfloat16)
    bproj16 = b_proj.reshape(1, DIM).astype(ml_dtypes.bfloat16)
    ones16 = np.ones((1, 128), ml_dtypes.bfloat16)
    ident = np.eye(128, dtype=np.float32).astype(ml_dtypes.bfloat16)

    in_maps = []
    for c in range(NCORES):
        xs = x[c * BPC:(c + 1) * BPC]                       # [2, N, DIM]
        xT = _round13(np.ascontiguousarray(xs.transpose(0, 2, 1)))
        in_maps.append({
            "xT": xT, "wqkvT": wqkvT, "wprojT": wprojT16,
            "bproj": bproj16, "cos2": cos2, "sin2": sin2,
            "ident": ident, "ones": ones16,
        })

    trace = bool(os.environ.get("KERNEL_TRACE"))
    kwargs = {}
    if trace:
        kwargs["trace"] = True
        td = os.environ.get("KERNEL_TRACE_DIR")
        if td:
            kwargs["tmpdir"] = td
    res = bass_utils.run_bass_kernel_spmd(
        nc, in_maps, core_ids=list(range(NCORES)), **kwargs)
    last_exec_time_ns = res.exec_time_ns
    out = np.concatenate([res.results[c]["y"] for c in range(NCORES)], axis=0)
    return np.ascontiguousarray(out.reshape(B, N, DIM).astype(np.float32))


if __name__ == "__main__":
    rng = np.random.default_rng(0)
    xs = rng.standard_normal((B, N, DIM), dtype=np.float32)
    wq = rng.standard_normal((3 * DIM, DIM), dtype=np.float32) / 32
    wp = rng.standard_normal((DIM, DIM), dtype=np.float32) / 32
    bp = np.zeros(DIM, np.float32)
    y = kernel(xs, wq, wp, bp)
    print("y", y.shape, y.dtype, float(np.abs(y).max()))


# revision 30
# speedup vs baseline: 1.0114x; 1.0048x over previous
"""Trainium2 Bass kernel for nn_Attention_13348758356565.

Dense transformer attention block (B=16, N=1024 tokens, DIM=1024, 16 heads x 64)
with axial rotary embeddings, data-parallel over batch across 8 NeuronCores
(2 batches per core).

v1 rewrite vs baseline: cross-batch pipelining (batch b+1's QKV matmuls fill
the PE during batch b's ACT-bound attention), exp batched to [128,1024] psum
reads (halves ACT instruction count), bf16 rotary in 3 DVE ops using a
negative-stride pair-swap access pattern with the rotation sign folded into
the sin table, bf16 PE transposes, and a PSUM budget of exactly 8 banks
(st 4 + pv 2 + matmul-stream 2).
"""

import os
import sys

sys.path.insert(0, "/opt/trn_rl_repo")

import dataclasses
import numpy as np

import concourse.bacc as bacc
import concourse.mybir as mybir
import concourse.tile as tile
from concourse import bass_utils

F32 = mybir.dt.float32
F32R = mybir.dt.float32r
BF16 = mybir.dt.bfloat16
EXP = mybir.ActivationFunctionType.Exp

B, HF, WF = 16, 32, 32
DIM, NH, HD = 1024, 16, 64
N = HF * WF          # 1024 tokens
NCORES = 8
BPC = B // NCORES    # 2 batches per core
ROT = HD // 2        # 32 rotary dims per head
SCALE = 1.0 / np.sqrt(HD)

NT = N // 128        # 8 token tiles
ND = DIM // 128      # 8 contraction tiles
NP = NH // 2         # 8 head pairs

last_exec_time_ns = None


def _round13(x):
    """Round fp32 mantissa to 13 bits (safe operand form for f32r matmuls)."""
    xi = np.ascontiguousarray(x, np.float32).view(np.uint32)
    xi = ((xi.astype(np.uint64) + (1 << 9)) >> 10 << 10).astype(np.uint32)
    return xi.view(np.float32)


def _freq_tables():
    """cos/sin expanded to the 32 rotary dims with rotate-half sign folded in.

    C2[n, k] = cos(f[n, k//2]);  S2[n, 2i] = -sin(f[n, i]), S2[n, 2i+1] = +sin.
    Rotation: out[k] = q[k]*C2[k] + q[k^1]*S2[k].
    """
    d = HD // 4
    base = (np.linspace(1.0, (HF * WF) / 2.0, d // 2, dtype=np.float64) * np.pi)
    posH = np.linspace(-1.0, 1.0, HF)
    posW = np.linspace(-1.0, 1.0, WF)
    fH = np.repeat(posH[:, None] * base[None, :], 2, axis=-1)   # [H, 16]
    fW = np.repeat(posW[:, None] * base[None, :], 2, axis=-1)   # [W, 16]
    fH = np.broadcast_to(fH[:, None, :], (HF, WF, d))
    fW = np.broadcast_to(fW[None, :, :], (HF, WF, d))
    freqs = np.concatenate([fH, fW], axis=-1).reshape(N, ROT)
    half = freqs[:, 0::2]                                       # [N, 16] pairs
    cos_h = np.cos(half)
    sin_h = np.sin(half)
    C2 = np.repeat(cos_h, 2, axis=1)                            # [N, 32]
    S2 = np.empty((N, ROT), np.float64)
    S2[:, 0::2] = -sin_h
    S2[:, 1::2] = sin_h
    return C2.astype(np.float32), S2.astype(np.float32)


def _bcast_mid(ap, count):
    """Insert a step-0 (broadcast) dim into AP after the partition dim."""
    return dataclasses.replace(ap, ap=[ap.ap[0], [0, count]] + list(ap.ap[1:]))


def _build():
    nc = bacc.Bacc("TRN2", target_bir_lowering=False, debug=False)

    xT_d = nc.dram_tensor("xT", [BPC, DIM, N], F32R, kind="ExternalInput")
    wqkvT_d = nc.dram_tensor("wqkvT", [DIM, 3 * DIM], F32R, kind="ExternalInput")
    wprojT_d = nc.dram_tensor("wprojT", [DIM, DIM], BF16, kind="ExternalInput")
    bproj_d = nc.dram_tensor("bproj", [1, DIM], BF16, kind="ExternalInput")
    cos2_d = nc.dram_tensor("cos2", [N, ROT], BF16, kind="ExternalInput")
    sin2_d = nc.dram_tensor("sin2", [N, ROT], BF16, kind="ExternalInput")
    ident_d = nc.dram_tensor("ident", [128, 128], BF16, kind="ExternalInput")
    ones_d = nc.dram_tensor("ones", [1, 128], BF16, kind="ExternalInput")
    y_d = nc.dram_tensor("y", [BPC, N, DIM], F32, kind="ExternalOutput")

    mul = mybir.AluOpType.mult
    add = mybir.AluOpType.add

    with tile.TileContext(nc) as tc:
        with (
            tc.tile_pool(name="sb", bufs=1) as sb,
            tc.tile_pool(name="ps", bufs=1, space="PSUM") as ps,
        ):
            # ---- constants ----
            ident = sb.tile([128, 128], BF16, name="ident")
            nc.sync.dma_start(ident[:], ident_d.ap())
            ones_r = sb.tile([1, 128], BF16, name="ones_r")
            nc.sync.dma_start(ones_r[:], ones_d.ap())
            bproj = sb.tile([1, DIM], BF16, name="bproj")
            nc.sync.dma_start(bproj[:], bproj_d.ap())
            # rotary tables: [128 tok-in-tile, t, 32]
            cos2 = sb.tile([128, NT * ROT], BF16, name="cos2")
            sin2 = sb.tile([128, NT * ROT], BF16, name="sin2")
            nc.sync.dma_start(
                cos2[:].rearrange("p (t c) -> p t c", c=ROT),
                cos2_d.ap().rearrange("(t p) c -> p t c", p=128),
            )
            nc.sync.dma_start(
                sin2[:].rearrange("p (t c) -> p t c", c=ROT),
                sin2_d.ap().rearrange("(t p) c -> p t c", p=128),
            )
            # wproj isn't needed until the first projection (~200us in); load
            # late so the startup DMA window is spent on x and w_qkv
            wprojT = [sb.tile([128, DIM], BF16, name=f"wprojT{d}") for d in range(ND)]
            wproj_loaded = [False]

            def load_wproj():
                if not wproj_loaded[0]:
                    wproj_loaded[0] = True
                    for d in range(ND):
                        nc.sync.dma_start(
                            wprojT[d][:], wprojT_d.ap()[d * 128:(d + 1) * 128, :])

            # HAM warm-up: ~5us of back-to-back dummy matmuls during the
            # initial x/w DMA wait so real work starts at the 2.4 GHz clock
            warm = ps.tile([128, 128], F32, name="warm", tag="mmq", bufs=2)
            for _ in range(48):
                nc.tensor.matmul(warm[:], ident[:], ident[:])

            def qkv_chunk(b, xT, tag, j0, consume):
                """qkv[:, j0:j0+512] for batch b; consume(t, pq) eats each
                [128, 512] psum tile of token-tile t."""
                wq = [sb.tile([128, 512], F32R, name=f"wq_{tag}_{d}",
                              tag=f"wq{d}", bufs=2) for d in range(ND)]
                for d in range(ND):
                    nc.sync.dma_start(
                        wq[d][:], wqkvT_d.ap()[d * 128:(d + 1) * 128, j0:j0 + 512])
                for t in range(NT):
                    pq = ps.tile([128, 512], F32, name=f"pq_{tag}_{t}",
                                 tag="mmq", bufs=2)
                    for d in range(ND):
                        nc.tensor.matmul(
                            pq[:], xT[d][:, t * 128:(t + 1) * 128], wq[d][:],
                            start=(d == 0), stop=(d == ND - 1))
                    consume(t, pq)

            def rotary(tag, t, pq):
                """Evacuate psum -> bf16 sbuf and rotate first 32 dims of each
                of the 8 heads in place: out[k] = q[k]*C2 + q[k^1]*S2."""
                qn = sb.tile([128, 512], BF16, name=f"qn_{tag}_{t}",
                             tag="qn", bufs=8)
                nc.vector.tensor_copy(qn[:], pq[:])   # frees the psum slot
                v4 = qn[:].rearrange("p (h i u) -> p h i u", h=8, i=32, u=2)
                rot = v4[:, :, 0:16, :]               # [128, 8, 16, 2]
                swp = v4[:, :, 0:16, ::-1]            # pair-swapped read
                cb = _bcast_mid(
                    cos2[:, t * ROT:(t + 1) * ROT]
                    .rearrange("p (i u) -> p i u", u=2), 8)
                sbb = _bcast_mid(
                    sin2[:, t * ROT:(t + 1) * ROT]
                    .rearrange("p (i u) -> p i u", u=2), 8)
                t1 = sb.tile([128, 8, 16, 2], BF16, name=f"t1_{tag}_{t}",
                             tag="rt1", bufs=2)
                t2 = sb.tile([128, 8, 16, 2], BF16, name=f"t2_{tag}_{t}",
                             tag="rt2", bufs=2)
                nc.vector.tensor_tensor(t1[:], rot, cb, mul)
                nc.vector.tensor_tensor(t2[:], swp, sbb, mul)
                nc.vector.tensor_tensor(rot, t1[:], t2[:], add)
                return qn

            def transpose_group(tag, qn4, grp, jt, dst):
                """PE-transpose col jt of 4 natural tiles into dst[:, grp*512:]."""
                tp = ps.tile([128, 512], BF16, name=f"tp_{tag}_{jt}_{grp}",
                             tag="mmq", bufs=2)
                for u in range(4):
                    nc.tensor.transpose(
                        tp[:, u * 128:(u + 1) * 128],
                        qn4[u][:, jt * 128:(jt + 1) * 128], ident[:])
                nc.vector.tensor_copy(dst[:, grp * 512:(grp + 1) * 512], tp[:])

            pending = []

            def flush_pending():
                for fn in pending:
                    fn()
                pending.clear()

            def attention(b, hp, kT, qT, vsb, outT):
                """One head pair: scores in [128,1024] psum (A|B), one exp,
                PV with ones-column denominators, normalize into outT.

                The last LAG PV matmuls and the normalize chain are deferred
                into the next (hp, nch) block so the next pair's independent
                QK matmuls sit between them in the PE queue instead of the PE
                head-of-line blocking on the tail exps."""
                LAG = 2
                hA, hB = 2 * hp, 2 * hp + 1
                for nch in range(2):
                    box = {}
                    pts = {}

                    def emit_pv(m, box=box, pts=pts, b=b, hp=hp, nch=nch,
                                hA=hA, hB=hB, vsb=vsb):
                        if "pvA" not in box:
                            box["pvA"] = ps.tile(
                                [HD + 1, 512], F32, name=f"pvA_b{b}_{hp}_{nch}",
                                tag="pv", bufs=2)
                            box["pvB"] = ps.tile(
                                [HD + 1, 512], F32, name=f"pvB_b{b}_{hp}_{nch}",
                                tag="pv", bufs=2)
                        pt = pts.pop(m)
                        nc.tensor.matmul(
                            box["pvA"][:], vsb[m][:, hA * (HD + 1):(hA + 1) * (HD + 1)],
                            pt[:, 0:512], start=(m == 0), stop=(m == NT - 1))
                        nc.tensor.matmul(
                            box["pvB"][:], vsb[m][:, hB * (HD + 1):(hB + 1) * (HD + 1)],
                            pt[:, 512:1024], start=(m == 0), stop=(m == NT - 1))

                    def normalize(box=box, b=b, hp=hp, nch=nch, outT=outT):
                        for half, pvx in ((0, box["pvA"]), (1, box["pvB"])):
                            # evacuate psum fast (frees the pv bank for the
                            # next pair), then normalize from sbuf
                            pvE = sb.tile([HD + 1, 512], F32,
                                          name=f"pvE_b{b}_{hp}_{nch}_{half}",
                                          tag="pvE", bufs=2)
                            nc.vector.tensor_copy(pvE[:], pvx[:])
                            dr = sb.tile([1, 512], F32,
                                         name=f"dr_b{b}_{hp}_{nch}_{half}",
                                         tag="dr", bufs=2)
                            nc.vector.tensor_copy(dr[:], pvE[64:65, :])
                            rr = sb.tile([1, 512], F32,
                                         name=f"rr_b{b}_{hp}_{nch}_{half}",
                                         tag="rr", bufs=2)
                            nc.vector.reciprocal_approx_fast(rr[:], dr[:])
                            rb = sb.tile([64, 512], F32,
                                         name=f"rb_b{b}_{hp}_{nch}_{half}",
                                         tag="rb", bufs=2)
                            nc.gpsimd.partition_broadcast(rb[:], rr[:])
                            nc.vector.tensor_tensor(
                                outT[hp][half * 64:half * 64 + 64,
                                         nch * 512:(nch + 1) * 512],
                                pvE[0:64, :], rb[:], mul)

                    for m in range(NT):
                        st = ps.tile([128, 1024], F32, name=f"st_b{b}_{hp}_{nch}_{m}",
                                     tag="st", bufs=2)
                        nc.tensor.matmul(
                            st[:, 0:512],
                            kT[hp][0:64, m * 128:(m + 1) * 128],
                            qT[hp][0:64, nch * 512:(nch + 1) * 512])
                        nc.tensor.matmul(
                            st[:, 512:1024],
                            kT[hp][64:128, m * 128:(m + 1) * 128],
                            qT[hp][64:128, nch * 512:(nch + 1) * 512])
                        pt = sb.tile([128, 1024], BF16, name=f"pt_b{b}_{hp}_{nch}_{m}",
                                     tag="pt", bufs=6)
                        nc.scalar.activation(pt[:], st[:], EXP, scale=float(SCALE))
                        pts[m] = pt
                        if m == 0:
                            flush_pending()
                        if m >= LAG:
                            emit_pv(m - LAG)
                    for m in range(NT - LAG, NT):
                        pending.append(lambda m=m, emit_pv=emit_pv: emit_pv(m))
                    pending.append(normalize)

            for b in range(BPC):
                xT = []
                for d in range(ND):
                    xt = sb.tile([128, N], F32R, name=f"xT_b{b}_{d}", tag=f"xT{d}")
                    nc.sync.dma_start(xt[:], xT_d.ap()[b, d * 128:(d + 1) * 128, :])
                    xT.append(xt)

                vsb = [sb.tile([128, NH * (HD + 1)], BF16, name=f"v_b{b}_{t}",
                               tag=f"v{t}", bufs=2) for t in range(NT)]
                for t in range(NT):
                    nc.vector.memset(
                        vsb[t][:].rearrange("p (h c) -> p h c", c=HD + 1)
                        [:, :, HD:], 1.0)

                qT = [sb.tile([128, N], BF16, name=f"qT_b{b}_{p}", tag=f"qT{p}")
                      for p in range(NP)]
                kT = [sb.tile([128, N], BF16, name=f"kT_b{b}_{p}", tag=f"kT{p}")
                      for p in range(NP)]
                outT = [sb.tile([128, N], BF16, name=f"outT_b{b}_{p}",
                                tag=f"outT{p}") for p in range(NP)]

                # ---- V first (it gates every head's PV) ----
                for jc in range(2):
                    def eat_v(t, pq, jc=jc):
                        h0 = jc * 8
                        nc.vector.tensor_copy(
                            vsb[t][:].rearrange("p (h c) -> p h c", c=HD + 1)
                            [:, h0:h0 + 8, 0:HD],
                            pq[:].rearrange("p (h c) -> p h c", c=HD))
                    qkv_chunk(b, xT, f"v{b}{jc}", 2 * DIM + jc * 512, eat_v)

                # ---- per group of 4 pairs: K chunk, Q chunk, attention ----
                for g in range(2):
                    for sect, dst_all in ((1, kT), (0, qT)):
                        qn_tiles = []

                        def eat_qk(t, pq, sect=sect, g=g, qn_tiles=qn_tiles):
                            qn_tiles.append(rotary(f"s{sect}b{b}g{g}", t, pq))
                            if t % 4 == 3:
                                grp = t // 4
                                for jt in range(4):
                                    transpose_group(
                                        f"s{sect}b{b}g{g}", qn_tiles[grp * 4:],
                                        grp, jt, dst_all[4 * g + jt])
                        qkv_chunk(b, xT, f"s{sect}b{b}g{g}",
                                  sect * DIM + g * 512, eat_qk)
                    for hp in range(4 * g, 4 * g + 4):
                        attention(b, hp, kT, qT, vsb, outT)

                flush_pending()
                load_wproj()

                # ---- output projection ----
                for t in range(NT):
                    for ec in range(2):
                        py = ps.tile([128, 512], F32, name=f"py_b{b}_{t}_{ec}",
                                     tag="mmq", bufs=2)
                        for d in range(ND):
                            nc.tensor.matmul(
                                py[:],
                                outT[d][:, t * 128:(t + 1) * 128],
                                wprojT[d][:, ec * 512:(ec + 1) * 512],
                                start=(d == 0), stop=False,
                            )
                        nc.tensor.matmul(
                            py[:], ones_r[:], bproj[:, ec * 512:(ec + 1) * 512],
                            start=False, stop=True,
                        )
                        ysb = sb.tile([128, 512], F32, name=f"y_b{b}_{t}_{ec}",
                                      tag="ysb", bufs=2)
                        nc.vector.tensor_copy(ysb[:], py[:])
                        nc.sync.dma_start(
                            y_d.ap()[b, t * 128:(t + 1) * 128,
                                     ec * 512:(ec + 1) * 512],
                            ysb[:],
                        )

    nc.compile()
    return nc


_NC_CACHE = None


def kernel(x, w_qkv, w_proj, b_proj):
    global _NC_CACHE, last_exec_time_ns
    x = np.ascontiguousarray(np.asarray(x, np.float32))
    w_qkv = np.asarray(w_qkv, np.float32)
    w_proj = np.asarray(w_proj, np.float32)
    b_proj = np.asarray(b_proj, np.float32)

    if _NC_CACHE is None:
        _NC_CACHE = _build()
    nc = _NC_CACHE

    import ml_dtypes
    C2, S2 = _freq_tables()
    cos2 = C2.astype(ml_dtypes.bfloat16)
    sin2 = S2.astype(ml_dtypes.bfloat16)
    wqkvT = _round13(np.ascontiguousarray(w_qkv.T))
    wprojT16 = np.ascontiguousarray(w_proj.T).astype(ml_dtypes.bfloat16)
    bproj16 = b_proj.reshape(1, DIM).astype(ml_dtypes.bfloat16)
    ones16 = np.ones((1, 128), ml_dtypes.bfloat16)
    ident = np.eye(128, dtype=np.float32).astype(ml_dtypes.bfloat16)

    in_maps = []
    for c in range(NCORES):
        xs = x[c * BPC:(c + 1) * BPC]                       # [2, N, DIM]
        xT = _round13(np.ascontiguousarray(xs.transpose(0, 2, 1)))
        in_maps.append({
            "xT": xT, "wqkvT": wqkvT, "wprojT": wprojT16,
            "bproj": bproj16, "cos2": cos2, "sin2": sin2,
            "ident": ident, "ones": ones16,
        })

    trace = bool(os.environ.get("KERNEL_TRACE"))
    kwargs = {}
    if trace:
        kwargs["trace"] = True
        td = os.environ.get("KERNEL_TRACE_DIR")
        if td:
            kwargs["tmpdir"] = td
    res = bass_utils.run_bass_kernel_spmd(
        nc, in_maps, core_ids=list(range(NCORES)), **kwargs)
    last_exec_time_ns = res.exec_time_ns
    out = np.concatenate([res.results[c]["y"] for c in range(NCORES)], axis=0)
    return np.ascontiguousarray(out.reshape(B, N, DIM).astype(np.float32))


if __name__ == "__main__":
    rng = np.random.default_rng(0)
    xs = rng.standard_normal((B, N, DIM), dtype=np.float32)
    wq = rng.standard_normal((3 * DIM, DIM), dtype=np.float32) / 32
    wp = rng.standard_normal((DIM, DIM), dtype=np.float32) / 32
    bp = np.zeros(DIM, np.float32)
    y = kernel(xs, wq, wp, bp)
    print("y", y.shape, y.dtype, float(np.abs(y).max()))
